# revision 9
# baseline (speedup 1.0000x reference)
"""BinaryDilGroupConv Trainium2 kernel.

Computes, for x[N=64, C=256, 32, 32]:
    h = BN(x)  (inference affine)
    a = sign(h); w = sign(weight)
    y = grouped dilated conv(a, w; groups=64, k=3, dil=2, pad=2)
    out = channel_shuffle(y, g=64) + x

Sharding: data-parallel over batch N across 8 NeuronCores (8 samples/core).
Params replicated. No collectives.

Device mapping (per core, per sample):
  - ACT: a = Sign(x*scale + bias) per 128-channel half, written bf16 into
    the interior of a zero-bordered padded tile [128, 36x36].
  - PE: grouped conv as block-diagonal matmuls: lhsT[h,tap] is [128 cin,
    128 cout] bf16 (zero off the 4x4 group diagonal). 9 dilation-shifted
    window reads of the padded tile accumulate into a PSUM tile per
    (half, 16-row chunk).
  - DVE: out_tile = psum + x_perm (x permuted by the inverse channel
    shuffle, built with SBUF->SBUF DMAs).
  - Store DMA scatters partitions to shuffled channel order in DRAM.
"""

import numpy as np
import ml_dtypes

C = 256
G = 64            # groups
CPG = 4           # channels per group
K = 3
DIL = 2
PAD = 2
EPS = 1e-5
H = W = 32
S = H * W         # 1024 spatial positions
PH = H + 2 * PAD  # padded rows (36)
PW = W + 2 * PAD  # padded cols (36)
N_FULL = 64
N_CORES = 8
NS = N_FULL // N_CORES   # samples per core
NHALF = 2                # channel halves of 128
CHUNK_ROWS = 16          # psum chunk = 16 rows x 32 cols = 512 <= 1 bank
ABUFS = 3                # padded-activation round-robin depth

_COMPILED = None


def build(n_samples=NS):
    """Build + compile the per-core Bass program."""
    import concourse.bacc as bacc
    import concourse.tile as tile
    import concourse.mybir as mybir

    fp32 = mybir.dt.float32
    bf16 = mybir.dt.bfloat16

    nc = bacc.Bacc("TRN2", target_bir_lowering=False, debug=False,
                   num_devices=N_CORES)

    xin = nc.dram_tensor("xin", [n_samples, NHALF, 128, S], fp32,
                         kind="ExternalInput").ap()
    wT = nc.dram_tensor("wT", [NHALF, K * K, 128, 128], bf16,
                        kind="ExternalInput").ap()
    bnsc = nc.dram_tensor("bnsc", [NHALF, 128], fp32,
                          kind="ExternalInput").ap()
    bnbi = nc.dram_tensor("bnbi", [NHALF, 128], fp32,
                          kind="ExternalInput").ap()
    out = nc.dram_tensor("out", [n_samples, C, S], fp32,
                         kind="ExternalOutput").ap()

    n_chunks = H // CHUNK_ROWS

    with tile.TileContext(nc) as tc:
        with (
            tc.tile_pool(name="const", bufs=1) as constp,
            tc.tile_pool(name="xp", bufs=3) as xp,
            tc.tile_pool(name="permp", bufs=3) as permp,
            tc.tile_pool(name="finp", bufs=3) as finp,
            tc.tile_pool(name="psum", bufs=8, space="PSUM") as psump,
        ):
            # ---- constants (loaded once) ----
            w_tile = constp.tile([128, NHALF * K * K, 128], bf16)
            nc.sync.dma_start(
                w_tile[:],
                wT.rearrange("h t k m -> k (h t) m"),
            )
            sc_tile = constp.tile([128, NHALF], fp32)
            nc.sync.dma_start(sc_tile[:], bnsc.rearrange("h p -> p h"))
            bi_tile = constp.tile([128, NHALF], fp32)
            nc.sync.dma_start(bi_tile[:], bnbi.rearrange("h p -> p h"))

            # ---- persistent padded activation tiles, borders zeroed once
            a_pads = [[constp.tile([128, PH * PW], bf16,
                                   name=f"apad{h}_{b}")
                       for b in range(ABUFS)] for h in range(NHALF)]
            for h in range(NHALF):
                for b in range(ABUFS):
                    ap3 = a_pads[h][b][:].rearrange("p (y x) -> p y x", x=PW)
                    nc.gpsimd.memset(ap3[:, 0:PAD, :], 0.0)
                    nc.gpsimd.memset(ap3[:, PAD + H:PH, :], 0.0)
                    nc.gpsimd.memset(ap3[:, PAD:PAD + H, 0:PAD], 0.0)
                    nc.gpsimd.memset(ap3[:, PAD:PAD + H, PAD + W:PW], 0.0)

            for n in range(n_samples):
                # ---- load x (partition = ch % 128, free = [half, pos])
                x_nat = xp.tile([128, NHALF, S], fp32)
                nc.sync.dma_start(x_nat[:], xin[n].rearrange("h p s -> p h s"))

                # PSUM partitions are ordered m = 32j + g (lhsT columns were
                # permuted on the host): partition m of half h holds conv
                # cout 128h + 4g + j, whose shuffled output channel is
                # f = 64j + 32h + g. So both the residual permute and the
                # store are contiguous 32-partition blocks.
                # ---- x_perm[32j + g, h, :] = x[64j + 32h + g]
                x_perm = permp.tile([128, NHALF, S], fp32)
                for h in range(NHALF):
                    for j in range(CPG):
                        src_lo = 64 * (j % 2) + 32 * h
                        nc.sync.dma_start(
                            x_perm[32 * j:32 * j + 32, h, :],
                            x_nat[src_lo:src_lo + 32, j // 2, :],
                        )

                # ---- a = Sign(x*scale + bias), bf16, into padded interior
                for h in range(NHALF):
                    ap3 = a_pads[h][n % ABUFS][:].rearrange(
                        "p (y x) -> p y x", x=PW)
                    nc.scalar.activation(
                        ap3[:, PAD:PAD + H, PAD:PAD + W],
                        x_nat[:, h, :].rearrange("p (y x) -> p y x", x=W),
                        mybir.ActivationFunctionType.Sign,
                        bias=bi_tile[:, h:h + 1],
                        scale=sc_tile[:, h:h + 1],
                    )

                # ---- conv: block-diag matmuls accumulating per chunk
                fin = finp.tile([128, NHALF, S], fp32)
                for h in range(NHALF):
                    ap3 = a_pads[h][n % ABUFS][:].rearrange(
                        "p (y x) -> p y x", x=PW)
                    for k in range(n_chunks):
                        ps = psump.tile([128, CHUNK_ROWS * W], fp32,
                                        name="ps", tag="ps")
                        for dy in range(K):
                            for dx in range(K):
                                t = dy * K + dx
                                nc.tensor.matmul(
                                    ps[:],
                                    w_tile[:, h * K * K + t, :],
                                    ap3[:,
                                        k * CHUNK_ROWS + DIL * dy:
                                        k * CHUNK_ROWS + DIL * dy
                                        + CHUNK_ROWS,
                                        DIL * dx:DIL * dx + W],
                                    start=(t == 0), stop=(t == K * K - 1),
                                )
                        # ---- evict: final = psum + x_perm
                        lo = k * CHUNK_ROWS * W
                        hi = lo + CHUNK_ROWS * W
                        nc.vector.tensor_add(
                            fin[:, h, lo:hi], ps[:], x_perm[:, h, lo:hi],
                        )

                # ---- store with channel-shuffle scatter:
                # src (m=(j,g), h, s) -> dram channel 64j + 32h + g
                out_jghs = out[n].rearrange("(j h g) s -> j g h s",
                                            j=CPG, h=NHALF)
                for j in range(CPG):
                    nc.sync.dma_start(
                        out_jghs[j],
                        fin[32 * j:32 * j + 32, :, :],
                    )

    nc.compile()
    return nc


def _host_prep(x, weight, gamma, beta, running_mean, running_var):
    """Precompute BN affine + block-diagonal signed weights."""
    inv = (gamma / np.sqrt(running_var + EPS)).astype(np.float32)
    bias = (beta - running_mean * inv).astype(np.float32)
    wsign = np.sign(weight).astype(np.float32)   # [256, 4, 3, 3]

    lhsT = np.zeros((NHALF, K * K, 128, 128), np.float32)
    # Column m of lhsT (-> PSUM partition m) holds cout co = 4*(m%32)+m//32
    # within the half, so PSUM partition order is m = 32j + g for conv
    # cout 4g + j (see the device-side comment on x_perm).
    m = np.arange(128)
    co = CPG * (m % 32) + m // 32
    gl = co // CPG
    for h in range(NHALF):
        for dy in range(K):
            for dx in range(K):
                t = dy * K + dx
                for kk in range(CPG):
                    lhsT[h, t, CPG * gl + kk, m] = wsign[128 * h + co, kk,
                                                         dy, dx]
    lhsT = lhsT.astype(ml_dtypes.bfloat16)
    sc = np.ascontiguousarray(inv.reshape(NHALF, 128))
    bi = np.ascontiguousarray(bias.reshape(NHALF, 128))
    return lhsT, sc, bi


def _get_compiled():
    global _COMPILED
    if _COMPILED is None:
        _COMPILED = build(NS)
    return _COMPILED


def make_in_maps(x, weight, gamma, beta, running_mean, running_var):
    lhsT, sc, bi = _host_prep(x, weight, gamma, beta, running_mean,
                              running_var)
    xs = np.ascontiguousarray(x.astype(np.float32).reshape(
        N_CORES, NS, NHALF, 128, S))
    return [
        {"xin": xs[i], "wT": lhsT, "bnsc": sc, "bnbi": bi}
        for i in range(N_CORES)
    ]


def kernel(x, weight, gamma, beta, running_mean, running_var):
    from concourse.bass_utils import run_bass_kernel_spmd

    nc = _get_compiled()
    in_maps = make_in_maps(np.asarray(x), np.asarray(weight),
                           np.asarray(gamma), np.asarray(beta),
                           np.asarray(running_mean), np.asarray(running_var))
    res = run_bass_kernel_spmd(nc, in_maps, list(range(N_CORES)))
    outs = [res.results[i]["out"].reshape(NS, C, H, W)
            for i in range(N_CORES)]
    return np.concatenate(outs, axis=0).astype(np.float32)


# revision 10
# speedup vs baseline: 1.0942x; 1.0942x over previous
"""BinaryDilGroupConv Trainium2 kernel.

Computes, for x[N=64, C=256, 32, 32]:
    h = BN(x)  (inference affine)
    a = sign(h); w = sign(weight)
    y = grouped dilated conv(a, w; groups=64, k=3, dil=2, pad=2)
    out = channel_shuffle(y, g=64) + x

Sharding: data-parallel over batch N across 8 NeuronCores (8 samples/core).
Params replicated. No collectives.

Device mapping (per core, per sample):
  - ACT: a = Sign(x*scale + bias) per 128-channel half, written bf16 into
    the interior of a zero-bordered padded tile [128, 36x36].
  - PE: grouped conv as block-diagonal matmuls: lhsT[h,tap] is [128 cin,
    128 cout] bf16 (zero off the 4x4 group diagonal). 9 dilation-shifted
    window reads of the padded tile accumulate into a PSUM tile per
    (half, 16-row chunk). PSUM partition order is m = 32j + g for conv
    cout 4g + j (lhsT columns permuted on the host) so the shuffle
    scatter below uses contiguous partition blocks.
  - DVE: evict PSUM to int8 (conv outputs are small integers, exact),
    DMA-permute the int8 conv tensor into shuffled (natural output)
    channel order (2MB instead of permuting 8MB of f32 x), then
    fin = x + conv_perm in natural layout.
  - Load and store are contiguous identity DMAs; the host pre/post
    reshapes (free) so DRAM runs are 8KB/partition.
"""

import numpy as np
import ml_dtypes

C = 256
G = 64            # groups
CPG = 4           # channels per group
K = 3
DIL = 2
PAD = 2
EPS = 1e-5
H = W = 32
S = H * W         # 1024 spatial positions
PH = H + 2 * PAD  # padded rows (36)
PW = W + 2 * PAD  # padded cols (36)
N_FULL = 64
N_CORES = 8
NS = N_FULL // N_CORES   # samples per core
NHALF = 2                # channel halves of 128
CHUNK_ROWS = 16          # psum chunk = 16 rows x 32 cols = 512 <= 1 bank
ABUFS = 3                # padded-activation round-robin depth
PREFETCH = 2             # x loads issued this many samples ahead

_COMPILED = None


def build(n_samples=NS):
    """Build + compile the per-core Bass program."""
    import concourse.bacc as bacc
    import concourse.tile as tile
    import concourse.mybir as mybir

    fp32 = mybir.dt.float32
    bf16 = mybir.dt.bfloat16
    i8 = mybir.dt.int8

    nc = bacc.Bacc("TRN2", target_bir_lowering=False, debug=False,
                   num_devices=N_CORES)

    # partition-major layouts so load/store DMAs are contiguous
    xin = nc.dram_tensor("xin", [n_samples, 128, NHALF, S], fp32,
                         kind="ExternalInput").ap()
    wT = nc.dram_tensor("wT", [NHALF, K * K, 128, 128], bf16,
                        kind="ExternalInput").ap()
    bnsc = nc.dram_tensor("bnsc", [NHALF, 128], fp32,
                          kind="ExternalInput").ap()
    bnbi = nc.dram_tensor("bnbi", [NHALF, 128], fp32,
                          kind="ExternalInput").ap()
    out = nc.dram_tensor("out", [n_samples, 128, NHALF, S], fp32,
                         kind="ExternalOutput").ap()

    n_chunks = H // CHUNK_ROWS

    with tile.TileContext(nc) as tc:
        with (
            tc.tile_pool(name="const", bufs=1) as constp,
            tc.tile_pool(name="xp", bufs=4) as xp,
            tc.tile_pool(name="ci8p", bufs=4) as ci8p,
            tc.tile_pool(name="cpermp", bufs=3) as cpermp,
            tc.tile_pool(name="finp", bufs=3) as finp,
            tc.tile_pool(name="psum", bufs=8, space="PSUM") as psump,
        ):
            # ---- constants (loaded once) ----
            w_tile = constp.tile([128, NHALF * K * K, 128], bf16)
            nc.sync.dma_start(
                w_tile[:],
                wT.rearrange("h t k m -> k (h t) m"),
            )
            sc_tile = constp.tile([128, NHALF], fp32)
            nc.sync.dma_start(sc_tile[:], bnsc.rearrange("h p -> p h"))
            bi_tile = constp.tile([128, NHALF], fp32)
            nc.sync.dma_start(bi_tile[:], bnbi.rearrange("h p -> p h"))

            # ---- persistent padded activation tiles, borders zeroed once
            a_pads = [[constp.tile([128, PH * PW], bf16,
                                   name=f"apad{h}_{b}")
                       for b in range(ABUFS)] for h in range(NHALF)]
            for h in range(NHALF):
                for b in range(ABUFS):
                    ap3 = a_pads[h][b][:].rearrange("p (y x) -> p y x", x=PW)
                    nc.gpsimd.memset(ap3[:, 0:PAD, :], 0.0)
                    nc.gpsimd.memset(ap3[:, PAD + H:PH, :], 0.0)
                    nc.gpsimd.memset(ap3[:, PAD:PAD + H, 0:PAD], 0.0)
                    nc.gpsimd.memset(ap3[:, PAD:PAD + H, PAD + W:PW], 0.0)

            # ---- x loads, prefetched ahead of the compute pipeline ----
            x_nats = {}

            def load_x(n):
                x_nats[n] = xp.tile([128, NHALF, S], fp32, name="x_nat",
                                    tag="x_nat")
                nc.sync.dma_start(x_nats[n][:], xin[n])

            for n in range(min(PREFETCH, n_samples)):
                load_x(n)

            for n in range(n_samples):
                if n + PREFETCH < n_samples:
                    load_x(n + PREFETCH)
                x_nat = x_nats.pop(n)

                # ---- a = Sign(x*scale + bias), bf16, into padded interior
                for h in range(NHALF):
                    ap3 = a_pads[h][n % ABUFS][:].rearrange(
                        "p (y x) -> p y x", x=PW)
                    nc.scalar.activation(
                        ap3[:, PAD:PAD + H, PAD:PAD + W],
                        x_nat[:, h, :].rearrange("p (y x) -> p y x", x=W),
                        mybir.ActivationFunctionType.Sign,
                        bias=bi_tile[:, h:h + 1],
                        scale=sc_tile[:, h:h + 1],
                    )

                # ---- conv: block-diag matmuls accumulating per chunk,
                # evicted to int8 (exact: conv values are small ints)
                conv_i8 = [ci8p.tile([128, S], i8, name=f"ci8_{h}",
                                     tag=f"ci8_{h}") for h in range(NHALF)]
                for h in range(NHALF):
                    ap3 = a_pads[h][n % ABUFS][:].rearrange(
                        "p (y x) -> p y x", x=PW)
                    for k in range(n_chunks):
                        ps = psump.tile([128, CHUNK_ROWS * W], fp32,
                                        name="ps", tag="ps")
                        for dy in range(K):
                            for dx in range(K):
                                t = dy * K + dx
                                nc.tensor.matmul(
                                    ps[:],
                                    w_tile[:, h * K * K + t, :],
                                    ap3[:,
                                        k * CHUNK_ROWS + DIL * dy:
                                        k * CHUNK_ROWS + DIL * dy
                                        + CHUNK_ROWS,
                                        DIL * dx:DIL * dx + W],
                                    start=(t == 0), stop=(t == K * K - 1),
                                )
                        lo = k * CHUNK_ROWS * W
                        nc.vector.tensor_copy(
                            conv_i8[h][:, lo:lo + CHUNK_ROWS * W], ps[:])

                # ---- shuffle-permute the int8 conv into natural final
                # channel order: psum (m=32j+g, half h) holds conv cout
                # 4g+j -> final channel 64j+32h+g = (slot j//2,
                # partition 64*(j%2)+32h+g).
                conv_perm = cpermp.tile([128, NHALF, S], i8)
                for h in range(NHALF):
                    for j in range(CPG):
                        eng = nc.gpsimd if h == 0 else nc.scalar
                        eng.dma_start(
                            conv_perm[64 * (j % 2) + 32 * h:
                                      64 * (j % 2) + 32 * h + 32,
                                      j // 2, :],
                            conv_i8[h][32 * j:32 * j + 32, :],
                        )

                # ---- residual add in natural layout, then identity store
                fin = finp.tile([128, NHALF, S], fp32)
                for hh in range(NHALF):
                    nc.vector.tensor_add(
                        fin[:, hh, :], x_nat[:, hh, :], conv_perm[:, hh, :])
                nc.sync.dma_start(out[n], fin[:])

    nc.compile()
    return nc


def _host_prep(x, weight, gamma, beta, running_mean, running_var):
    """Precompute BN affine + block-diagonal signed weights."""
    inv = (gamma / np.sqrt(running_var + EPS)).astype(np.float32)
    bias = (beta - running_mean * inv).astype(np.float32)
    wsign = np.sign(weight).astype(np.float32)   # [256, 4, 3, 3]

    lhsT = np.zeros((NHALF, K * K, 128, 128), np.float32)
    # Column m of lhsT (-> PSUM partition m) holds cout co = 4*(m%32)+m//32
    # within the half, so PSUM partition order is m = 32j + g for conv
    # cout 4g + j (see the device-side comment on conv_perm).
    m = np.arange(128)
    co = CPG * (m % 32) + m // 32
    gl = co // CPG
    for h in range(NHALF):
        for dy in range(K):
            for dx in range(K):
                t = dy * K + dx
                for kk in range(CPG):
                    lhsT[h, t, CPG * gl + kk, m] = wsign[128 * h + co, kk,
                                                         dy, dx]
    lhsT = lhsT.astype(ml_dtypes.bfloat16)
    sc = np.ascontiguousarray(inv.reshape(NHALF, 128))
    bi = np.ascontiguousarray(bias.reshape(NHALF, 128))
    return lhsT, sc, bi


def _get_compiled():
    global _COMPILED
    if _COMPILED is None:
        _COMPILED = build(NS)
    return _COMPILED


def make_in_maps(x, weight, gamma, beta, running_mean, running_var):
    lhsT, sc, bi = _host_prep(x, weight, gamma, beta, running_mean,
                              running_var)
    # [cores, ns, 2, 128, S] -> partition-major [cores, ns, 128, 2, S]
    xs = np.ascontiguousarray(
        x.astype(np.float32)
        .reshape(N_CORES, NS, NHALF, 128, S)
        .transpose(0, 1, 3, 2, 4))
    return [
        {"xin": xs[i], "wT": lhsT, "bnsc": sc, "bnbi": bi}
        for i in range(N_CORES)
    ]


def kernel(x, weight, gamma, beta, running_mean, running_var):
    from concourse.bass_utils import run_bass_kernel_spmd

    nc = _get_compiled()
    in_maps = make_in_maps(np.asarray(x), np.asarray(weight),
                           np.asarray(gamma), np.asarray(beta),
                           np.asarray(running_mean), np.asarray(running_var))
    res = run_bass_kernel_spmd(nc, in_maps, list(range(N_CORES)))
    # device out is [ns, 128, 2, S] partition-major; channel c' = 128*slot+p
    outs = [res.results[i]["out"].transpose(0, 2, 1, 3).reshape(NS, C, H, W)
            for i in range(N_CORES)]
    return np.concatenate(outs, axis=0).astype(np.float32)


# revision 15
# speedup vs baseline: 1.1214x; 1.0248x over previous
"""BinaryDilGroupConv Trainium2 kernel.

Computes, for x[N=64, C=256, 32, 32]:
    h = BN(x)  (inference affine)
    a = sign(h); w = sign(weight)
    y = grouped dilated conv(a, w; groups=64, k=3, dil=2, pad=2)
    out = channel_shuffle(y, g=64) + x

Sharding: data-parallel over batch N across 8 NeuronCores (8 samples/core).
Params replicated. No collectives.

Device mapping (per core, per sample):
  - ACT: a = Sign(x*scale + bias) per 128-channel half, written bf16 into
    the interior of a zero-bordered padded tile [128, 36x36].
  - PE: grouped conv as block-diagonal matmuls: lhsT[h,tap] is [128 cin,
    128 cout] bf16 (zero off the 4x4 group diagonal). 9 dilation-shifted
    window reads of the padded tile accumulate into a PSUM tile per
    (half, 16-row chunk). PSUM partition order is m = 32j + g for conv
    cout 4g + j (lhsT columns permuted on the host) so the shuffle
    scatter below uses contiguous partition blocks.
  - DVE: evict PSUM to int8 (conv outputs are small integers, exact),
    DMA-permute the int8 conv tensor into shuffled (natural output)
    channel order (2MB instead of permuting 8MB of f32 x), then
    fin = x + conv_perm in natural layout.
  - Load and store are contiguous identity DMAs; the host pre/post
    reshapes (free) so DRAM runs are 8KB/partition.
"""

import numpy as np
import ml_dtypes

C = 256
G = 64            # groups
CPG = 4           # channels per group
K = 3
DIL = 2
PAD = 2
EPS = 1e-5
H = W = 32
S = H * W         # 1024 spatial positions
PH = H + 2 * PAD  # padded rows (36)
PW = W + 2 * PAD  # padded cols (36)
N_FULL = 64
N_CORES = 8
NS = N_FULL // N_CORES   # samples per core
NHALF = 2                # channel halves of 128
CHUNK_ROWS = 16          # psum chunk = 16 rows x 32 cols = 512 <= 1 bank
ABUFS = 3                # padded-activation round-robin depth
PREFETCH = 2             # x loads issued this many samples ahead

_COMPILED = None


def build(n_samples=NS):
    """Build + compile the per-core Bass program."""
    import concourse.bacc as bacc
    import concourse.tile as tile
    import concourse.mybir as mybir

    fp32 = mybir.dt.float32
    bf16 = mybir.dt.bfloat16
    i8 = mybir.dt.int8

    nc = bacc.Bacc("TRN2", target_bir_lowering=False, debug=False,
                   num_devices=N_CORES)

    # partition-major layouts so load/store DMAs are contiguous
    xin = nc.dram_tensor("xin", [n_samples, 128, NHALF, S], fp32,
                         kind="ExternalInput").ap()
    wT = nc.dram_tensor("wT", [128, NHALF * K * K, 128], bf16,
                        kind="ExternalInput").ap()
    bnsc = nc.dram_tensor("bnsc", [NHALF, 128], fp32,
                          kind="ExternalInput").ap()
    bnbi = nc.dram_tensor("bnbi", [NHALF, 128], fp32,
                          kind="ExternalInput").ap()
    out = nc.dram_tensor("out", [n_samples, 128, NHALF, S], fp32,
                         kind="ExternalOutput").ap()

    n_chunks = H // CHUNK_ROWS

    with tile.TileContext(nc) as tc:
        with (
            tc.tile_pool(name="const", bufs=1) as constp,
            tc.tile_pool(name="xp", bufs=4) as xp,
            tc.tile_pool(name="ci8p", bufs=4) as ci8p,
            tc.tile_pool(name="cpermp", bufs=3) as cpermp,
            tc.tile_pool(name="finp", bufs=3) as finp,
            tc.tile_pool(name="psum", bufs=8, space="PSUM") as psump,
        ):
            # ---- constants; BN params + first x loads go first so the
            # first Sign starts ASAP, weights overlap with it
            sc_tile = constp.tile([128, NHALF], fp32)
            nc.sync.dma_start(sc_tile[:], bnsc.rearrange("h p -> p h"))
            bi_tile = constp.tile([128, NHALF], fp32)
            nc.sync.dma_start(bi_tile[:], bnbi.rearrange("h p -> p h"))

            # ---- persistent padded activation tiles, borders zeroed once
            a_pads = [[constp.tile([128, PH * PW], bf16,
                                   name=f"apad{h}_{b}")
                       for b in range(ABUFS)] for h in range(NHALF)]
            for h in range(NHALF):
                for b in range(ABUFS):
                    ap3 = a_pads[h][b][:].rearrange("p (y x) -> p y x", x=PW)
                    nc.gpsimd.memset(ap3[:, 0:PAD, :], 0.0)
                    nc.gpsimd.memset(ap3[:, PAD + H:PH, :], 0.0)
                    nc.gpsimd.memset(ap3[:, PAD:PAD + H, 0:PAD], 0.0)
                    nc.gpsimd.memset(ap3[:, PAD:PAD + H, PAD + W:PW], 0.0)

            # ---- x loads, prefetched ahead of the compute pipeline ----
            x_nats = {}

            def load_x(n):
                x_nats[n] = xp.tile([128, NHALF, S], fp32, name="x_nat",
                                    tag="x_nat")
                nc.sync.dma_start(x_nats[n][:], xin[n])

            for n in range(min(PREFETCH, n_samples)):
                load_x(n)

            w_tile = constp.tile([128, NHALF * K * K, 128], bf16)
            nc.sync.dma_start(w_tile[:], wT)

            for n in range(n_samples):
                if n + PREFETCH < n_samples:
                    load_x(n + PREFETCH)
                x_nat = x_nats.pop(n)

                # ---- a = Sign(x*scale + bias), bf16, into padded interior
                for h in range(NHALF):
                    ap3 = a_pads[h][n % ABUFS][:].rearrange(
                        "p (y x) -> p y x", x=PW)
                    nc.scalar.activation(
                        ap3[:, PAD:PAD + H, PAD:PAD + W],
                        x_nat[:, h, :].rearrange("p (y x) -> p y x", x=W),
                        mybir.ActivationFunctionType.Sign,
                        bias=bi_tile[:, h:h + 1],
                        scale=sc_tile[:, h:h + 1],
                    )

                # ---- conv: block-diag matmuls accumulating per chunk,
                # evicted to int8 (exact: conv values are small ints)
                conv_i8 = [ci8p.tile([128, S], i8, name=f"ci8_{h}",
                                     tag=f"ci8_{h}") for h in range(NHALF)]
                for h in range(NHALF):
                    ap3 = a_pads[h][n % ABUFS][:].rearrange(
                        "p (y x) -> p y x", x=PW)
                    for k in range(n_chunks):
                        ps = psump.tile([128, CHUNK_ROWS * W], fp32,
                                        name="ps", tag="ps")
                        for dy in range(K):
                            for dx in range(K):
                                t = dy * K + dx
                                nc.tensor.matmul(
                                    ps[:],
                                    w_tile[:, h * K * K + t, :],
                                    ap3[:,
                                        k * CHUNK_ROWS + DIL * dy:
                                        k * CHUNK_ROWS + DIL * dy
                                        + CHUNK_ROWS,
                                        DIL * dx:DIL * dx + W],
                                    start=(t == 0), stop=(t == K * K - 1),
                                )
                        lo = k * CHUNK_ROWS * W
                        nc.vector.tensor_copy(
                            conv_i8[h][:, lo:lo + CHUNK_ROWS * W], ps[:])

                # ---- shuffle-permute the int8 conv into natural final
                # channel order: psum (m=32j+g, half h) holds conv cout
                # 4g+j -> final channel 64j+32h+g = (slot j//2,
                # partition 64*(j%2)+32h+g).
                conv_perm = cpermp.tile([128, NHALF, S], i8)
                for h in range(NHALF):
                    for j in range(CPG):
                        nc.gpsimd.dma_start(
                            conv_perm[64 * (j % 2) + 32 * h:
                                      64 * (j % 2) + 32 * h + 32,
                                      j // 2, :],
                            conv_i8[h][32 * j:32 * j + 32, :],
                        )

                # ---- residual add in natural layout, then identity store
                fin = finp.tile([128, NHALF, S], fp32)
                for hh in range(NHALF):
                    nc.vector.tensor_add(
                        fin[:, hh, :], x_nat[:, hh, :], conv_perm[:, hh, :])
                nc.sync.dma_start(out[n], fin[:])

    nc.compile()
    return nc


def _host_prep(x, weight, gamma, beta, running_mean, running_var):
    """Precompute BN affine + block-diagonal signed weights."""
    inv = (gamma / np.sqrt(running_var + EPS)).astype(np.float32)
    bias = (beta - running_mean * inv).astype(np.float32)
    wsign = np.sign(weight).astype(np.float32)   # [256, 4, 3, 3]

    lhsT = np.zeros((NHALF, K * K, 128, 128), np.float32)
    # Column m of lhsT (-> PSUM partition m) holds cout co = 4*(m%32)+m//32
    # within the half, so PSUM partition order is m = 32j + g for conv
    # cout 4g + j (see the device-side comment on conv_perm).
    m = np.arange(128)
    co = CPG * (m % 32) + m // 32
    gl = co // CPG
    for h in range(NHALF):
        for dy in range(K):
            for dx in range(K):
                t = dy * K + dx
                for kk in range(CPG):
                    lhsT[h, t, CPG * gl + kk, m] = wsign[128 * h + co, kk,
                                                         dy, dx]
    # device weight layout: [ci, (h,t), m] so the upload is contiguous
    lhsT = np.ascontiguousarray(
        lhsT.astype(ml_dtypes.bfloat16)
        .transpose(2, 0, 1, 3)
        .reshape(128, NHALF * K * K, 128))
    sc = np.ascontiguousarray(inv.reshape(NHALF, 128))
    bi = np.ascontiguousarray(bias.reshape(NHALF, 128))
    return lhsT, sc, bi


def _get_compiled():
    global _COMPILED
    if _COMPILED is None:
        _COMPILED = build(NS)
    return _COMPILED


def make_in_maps(x, weight, gamma, beta, running_mean, running_var):
    lhsT, sc, bi = _host_prep(x, weight, gamma, beta, running_mean,
                              running_var)
    # [cores, ns, 2, 128, S] -> partition-major [cores, ns, 128, 2, S]
    xs = np.ascontiguousarray(
        x.astype(np.float32)
        .reshape(N_CORES, NS, NHALF, 128, S)
        .transpose(0, 1, 3, 2, 4))
    return [
        {"xin": xs[i], "wT": lhsT, "bnsc": sc, "bnbi": bi}
        for i in range(N_CORES)
    ]


def kernel(x, weight, gamma, beta, running_mean, running_var):
    from concourse.bass_utils import run_bass_kernel_spmd

    nc = _get_compiled()
    in_maps = make_in_maps(np.asarray(x), np.asarray(weight),
                           np.asarray(gamma), np.asarray(beta),
                           np.asarray(running_mean), np.asarray(running_var))
    res = run_bass_kernel_spmd(nc, in_maps, list(range(N_CORES)))
    # device out is [ns, 128, 2, S] partition-major; channel c' = 128*slot+p
    outs = [res.results[i]["out"].transpose(0, 2, 1, 3).reshape(NS, C, H, W)
            for i in range(N_CORES)]
    return np.concatenate(outs, axis=0).astype(np.float32)


# revision 19
# speedup vs baseline: 1.1789x; 1.0513x over previous
"""BinaryDilGroupConv Trainium2 kernel.

Computes, for x[N=64, C=256, 32, 32]:
    h = BN(x)  (inference affine)
    a = sign(h); w = sign(weight)
    y = grouped dilated conv(a, w; groups=64, k=3, dil=2, pad=2)
    out = channel_shuffle(y, g=64) + x

Sharding: data-parallel over batch N across 8 NeuronCores (8 samples/core).
Params replicated. No collectives.

Device mapping (per core, per sample):
  - ACT: a = Sign(x*scale + bias) per 128-channel half, written bf16 into
    the interior of a zero-bordered padded tile [128, 36x36].
  - PE: grouped conv as block-diagonal matmuls: lhsT[h,tap] is [128 cin,
    128 cout] bf16 (zero off the 4x4 group diagonal). 9 dilation-shifted
    window reads of the padded tile accumulate into a PSUM tile per
    (half, 16-row chunk). PSUM partition order is m = 32j + g for conv
    cout 4g + j (lhsT columns permuted on the host) so the shuffle
    scatter below uses contiguous partition blocks.
  - DVE: evict PSUM to int8 (conv outputs are small integers, exact),
    DMA-permute the int8 conv tensor into shuffled (natural output)
    channel order (2MB instead of permuting 8MB of f32 x), then
    fin = x + conv_perm in natural layout.
  - Load and store are contiguous identity DMAs; the host pre/post
    reshapes (free) so DRAM runs are 8KB/partition.
"""

import numpy as np
import ml_dtypes

C = 256
G = 64            # groups
CPG = 4           # channels per group
K = 3
DIL = 2
PAD = 2
EPS = 1e-5
H = W = 32
S = H * W         # 1024 spatial positions
PH = H + 2 * PAD  # padded rows (36)
PW = W + 2 * PAD  # padded cols (36)
N_FULL = 64
N_CORES = 8
NS = N_FULL // N_CORES   # samples per core
NHALF = 2                # channel halves of 128
CHUNK_ROWS = 16          # psum chunk = 16 rows x 32 cols = 512 <= 1 bank
ABUFS = 3                # padded-activation round-robin depth
PREFETCH = 3             # x loads issued this many samples ahead

_COMPILED = None


def build(n_samples=NS):
    """Build + compile the per-core Bass program."""
    import concourse.bacc as bacc
    import concourse.tile as tile
    import concourse.mybir as mybir

    fp32 = mybir.dt.float32
    bf16 = mybir.dt.bfloat16
    i8 = mybir.dt.int8

    nc = bacc.Bacc("TRN2", target_bir_lowering=False, debug=False,
                   num_devices=N_CORES)

    # partition-major layouts so load/store DMAs are contiguous
    xin = nc.dram_tensor("xin", [n_samples, 128, NHALF, S], fp32,
                         kind="ExternalInput").ap()
    wT = nc.dram_tensor("wT", [128, NHALF * K * K, 128], bf16,
                        kind="ExternalInput").ap()
    bnsc = nc.dram_tensor("bnsc", [NHALF, 128], fp32,
                          kind="ExternalInput").ap()
    bnbi = nc.dram_tensor("bnbi", [NHALF, 128], fp32,
                          kind="ExternalInput").ap()
    out = nc.dram_tensor("out", [n_samples, 128, NHALF, S], fp32,
                         kind="ExternalOutput").ap()

    n_chunks = H // CHUNK_ROWS

    with tile.TileContext(nc) as tc:
        with (
            tc.tile_pool(name="const", bufs=1) as constp,
            tc.tile_pool(name="xp", bufs=5) as xp,
            tc.tile_pool(name="ci8p", bufs=4) as ci8p,
            tc.tile_pool(name="cpermp", bufs=3) as cpermp,
            tc.tile_pool(name="finp", bufs=3) as finp,
            tc.tile_pool(name="psum", bufs=8, space="PSUM") as psump,
        ):
            # ---- first x load + BN params go first so the first Sign
            # starts ASAP; weights overlap with it
            x_nats = {}

            def load_x(n):
                x_nats[n] = xp.tile([128, NHALF, S], fp32, name="x_nat",
                                    tag="x_nat")
                nc.sync.dma_start(x_nats[n][:], xin[n])

            load_x(0)
            sc_tile = constp.tile([128, NHALF], fp32)
            nc.sync.dma_start(sc_tile[:], bnsc.rearrange("h p -> p h"))
            bi_tile = constp.tile([128, NHALF], fp32)
            nc.sync.dma_start(bi_tile[:], bnbi.rearrange("h p -> p h"))

            # ---- persistent padded activation tiles, borders zeroed once
            a_pads = [[constp.tile([128, PH * PW], bf16,
                                   name=f"apad{h}_{b}")
                       for b in range(ABUFS)] for h in range(NHALF)]
            for h in range(NHALF):
                for b in range(ABUFS):
                    ap3 = a_pads[h][b][:].rearrange("p (y x) -> p y x", x=PW)
                    nc.gpsimd.memset(ap3[:, 0:PAD, :], 0.0)
                    nc.gpsimd.memset(ap3[:, PAD + H:PH, :], 0.0)
                    nc.gpsimd.memset(ap3[:, PAD:PAD + H, 0:PAD], 0.0)
                    nc.gpsimd.memset(ap3[:, PAD:PAD + H, PAD + W:PW], 0.0)

            # ---- remaining prefetches + weights ----
            for n in range(1, min(PREFETCH, n_samples)):
                load_x(n)

            w_tile = constp.tile([128, NHALF * K * K, 128], bf16)
            nc.sync.dma_start(w_tile[:], wT)

            # deferred-by-one-sample residual add + store, so the DVE/SP
            # FIFOs never make sample n+1's evictions wait on sample n's
            # permute chain
            deferred = {}

            def add_and_store(n):
                x_nat_n, conv_perm_n = deferred.pop(n)
                fin = finp.tile([128, NHALF, S], fp32, name="fin",
                                tag="fin")
                for hh in range(NHALF):
                    nc.vector.tensor_add(
                        fin[:, hh, :], x_nat_n[:, hh, :],
                        conv_perm_n[:, hh, :])
                    nc.sync.dma_start(out[n][:, hh, :], fin[:, hh, :])

            for n in range(n_samples):
                if n + PREFETCH < n_samples:
                    load_x(n + PREFETCH)
                x_nat = x_nats.pop(n)

                # ---- a = Sign(x*scale + bias), bf16, into padded interior
                for h in range(NHALF):
                    ap3 = a_pads[h][n % ABUFS][:].rearrange(
                        "p (y x) -> p y x", x=PW)
                    nc.scalar.activation(
                        ap3[:, PAD:PAD + H, PAD:PAD + W],
                        x_nat[:, h, :].rearrange("p (y x) -> p y x", x=W),
                        mybir.ActivationFunctionType.Sign,
                        bias=bi_tile[:, h:h + 1],
                        scale=sc_tile[:, h:h + 1],
                    )

                # ---- conv: block-diag matmuls accumulating per chunk,
                # evicted to int8 (exact: conv values are small ints)
                conv_i8 = [ci8p.tile([128, S], i8, name=f"ci8_{h}",
                                     tag=f"ci8_{h}") for h in range(NHALF)]
                for h in range(NHALF):
                    ap3 = a_pads[h][n % ABUFS][:].rearrange(
                        "p (y x) -> p y x", x=PW)
                    for k in range(n_chunks):
                        ps = psump.tile([128, CHUNK_ROWS * W], fp32,
                                        name="ps", tag="ps")
                        for dy in range(K):
                            for dx in range(K):
                                t = dy * K + dx
                                nc.tensor.matmul(
                                    ps[:],
                                    w_tile[:, h * K * K + t, :],
                                    ap3[:,
                                        k * CHUNK_ROWS + DIL * dy:
                                        k * CHUNK_ROWS + DIL * dy
                                        + CHUNK_ROWS,
                                        DIL * dx:DIL * dx + W],
                                    start=(t == 0), stop=(t == K * K - 1),
                                )
                        lo = k * CHUNK_ROWS * W
                        nc.vector.tensor_copy(
                            conv_i8[h][:, lo:lo + CHUNK_ROWS * W], ps[:])

                # ---- shuffle-permute the int8 conv into natural final
                # channel order: psum (m=32j+g, half h) holds conv cout
                # 4g+j -> final channel 64j+32h+g = (slot j//2,
                # partition 64*(j%2)+32h+g).
                conv_perm = cpermp.tile([128, NHALF, S], i8)
                for j in range(CPG):
                    for h in range(NHALF):
                        nc.gpsimd.dma_start(
                            conv_perm[64 * (j % 2) + 32 * h:
                                      64 * (j % 2) + 32 * h + 32,
                                      j // 2, :],
                            conv_i8[h][32 * j:32 * j + 32, :],
                        )

                # ---- residual add + store for the PREVIOUS sample ----
                deferred[n] = (x_nat, conv_perm)
                if n > 0:
                    add_and_store(n - 1)
            add_and_store(n_samples - 1)

    nc.compile()
    return nc


def _host_prep(x, weight, gamma, beta, running_mean, running_var):
    """Precompute BN affine + block-diagonal signed weights."""
    inv = (gamma / np.sqrt(running_var + EPS)).astype(np.float32)
    bias = (beta - running_mean * inv).astype(np.float32)
    wsign = np.sign(weight).astype(np.float32)   # [256, 4, 3, 3]

    lhsT = np.zeros((NHALF, K * K, 128, 128), np.float32)
    # Column m of lhsT (-> PSUM partition m) holds cout co = 4*(m%32)+m//32
    # within the half, so PSUM partition order is m = 32j + g for conv
    # cout 4g + j (see the device-side comment on conv_perm).
    m = np.arange(128)
    co = CPG * (m % 32) + m // 32
    gl = co // CPG
    for h in range(NHALF):
        for dy in range(K):
            for dx in range(K):
                t = dy * K + dx
                for kk in range(CPG):
                    lhsT[h, t, CPG * gl + kk, m] = wsign[128 * h + co, kk,
                                                         dy, dx]
    # device weight layout: [ci, (h,t), m] so the upload is contiguous
    lhsT = np.ascontiguousarray(
        lhsT.astype(ml_dtypes.bfloat16)
        .transpose(2, 0, 1, 3)
        .reshape(128, NHALF * K * K, 128))
    sc = np.ascontiguousarray(inv.reshape(NHALF, 128))
    bi = np.ascontiguousarray(bias.reshape(NHALF, 128))
    return lhsT, sc, bi


def _get_compiled():
    global _COMPILED
    if _COMPILED is None:
        _COMPILED = build(NS)
    return _COMPILED


def make_in_maps(x, weight, gamma, beta, running_mean, running_var):
    lhsT, sc, bi = _host_prep(x, weight, gamma, beta, running_mean,
                              running_var)
    # [cores, ns, 2, 128, S] -> partition-major [cores, ns, 128, 2, S]
    xs = np.ascontiguousarray(
        x.astype(np.float32)
        .reshape(N_CORES, NS, NHALF, 128, S)
        .transpose(0, 1, 3, 2, 4))
    return [
        {"xin": xs[i], "wT": lhsT, "bnsc": sc, "bnbi": bi}
        for i in range(N_CORES)
    ]


def kernel(x, weight, gamma, beta, running_mean, running_var):
    from concourse.bass_utils import run_bass_kernel_spmd

    nc = _get_compiled()
    in_maps = make_in_maps(np.asarray(x), np.asarray(weight),
                           np.asarray(gamma), np.asarray(beta),
                           np.asarray(running_mean), np.asarray(running_var))
    res = run_bass_kernel_spmd(nc, in_maps, list(range(N_CORES)))
    # device out is [ns, 128, 2, S] partition-major; channel c' = 128*slot+p
    outs = [res.results[i]["out"].transpose(0, 2, 1, 3).reshape(NS, C, H, W)
            for i in range(N_CORES)]
    return np.concatenate(outs, axis=0).astype(np.float32)


# revision 22
# speedup vs baseline: 1.2379x; 1.0501x over previous
"""BinaryDilGroupConv Trainium2 kernel.

Computes, for x[N=64, C=256, 32, 32]:
    h = BN(x)  (inference affine)
    a = sign(h); w = sign(weight)
    y = grouped dilated conv(a, w; groups=64, k=3, dil=2, pad=2)
    out = channel_shuffle(y, g=64) + x

Sharding: data-parallel over batch N across 8 NeuronCores (8 samples/core).
Params replicated. No collectives.

Device mapping (per core, per sample):
  - ACT: a = Sign(x*scale + bias) per 128-channel half, written bf16 into
    the interior of a zero-bordered padded tile [128, 36x36].
  - PE: grouped conv as block-diagonal matmuls: lhsT[h,tap] is [128 cin,
    128 cout] bf16 (zero off the 4x4 group diagonal). 9 dilation-shifted
    window reads of the padded tile accumulate into a PSUM tile per
    (half, 16-row chunk). PSUM partition order is m = 32j + g for conv
    cout 4g + j (lhsT columns permuted on the host) so the shuffle
    scatter below uses contiguous partition blocks.
  - DVE: evict PSUM to int8 (conv outputs are small integers, exact),
    DMA-permute the int8 conv tensor into shuffled (natural output)
    channel order (2MB instead of permuting 8MB of f32 x), then
    fin = x + conv_perm in natural layout.
  - Load and store are contiguous identity DMAs; the host pre/post
    reshapes (free) so DRAM runs are 8KB/partition.
"""

import numpy as np
import ml_dtypes

C = 256
G = 64            # groups
CPG = 4           # channels per group
K = 3
DIL = 2
PAD = 2
EPS = 1e-5
H = W = 32
S = H * W         # 1024 spatial positions
PH = H + 2 * PAD  # padded rows (36)
PW = W + 2 * PAD  # padded cols (36)
N_FULL = 64
N_CORES = 8
NS = N_FULL // N_CORES   # samples per core
NHALF = 2                # channel halves of 128
CHUNK_ROWS = 16          # psum chunk = 16 rows x 32 cols = 512 <= 1 bank
ABUFS = 3                # padded-activation round-robin depth
PREFETCH = 3             # x loads issued this many samples ahead

_COMPILED = None


def build(n_samples=NS):
    """Build + compile the per-core Bass program."""
    import concourse.bacc as bacc
    import concourse.tile as tile
    import concourse.mybir as mybir

    fp32 = mybir.dt.float32
    bf16 = mybir.dt.bfloat16
    i8 = mybir.dt.int8

    nc = bacc.Bacc("TRN2", target_bir_lowering=False, debug=False,
                   num_devices=N_CORES)

    # partition-major layouts so load/store DMAs are contiguous
    xin = nc.dram_tensor("xin", [n_samples, 128, NHALF, S], fp32,
                         kind="ExternalInput").ap()
    wT = nc.dram_tensor("wT", [128, NHALF * K * K, 128], bf16,
                        kind="ExternalInput").ap()
    bnsc = nc.dram_tensor("bnsc", [NHALF, 128], fp32,
                          kind="ExternalInput").ap()
    bnbi = nc.dram_tensor("bnbi", [NHALF, 128], fp32,
                          kind="ExternalInput").ap()
    out = nc.dram_tensor("out", [n_samples, 128, NHALF, S], fp32,
                         kind="ExternalOutput").ap()

    n_chunks = H // CHUNK_ROWS

    with tile.TileContext(nc) as tc:
        with (
            tc.tile_pool(name="const", bufs=1) as constp,
            tc.tile_pool(name="xp", bufs=5) as xp,
            tc.tile_pool(name="ci8p", bufs=4) as ci8p,
            tc.tile_pool(name="cpermp", bufs=3) as cpermp,
            tc.tile_pool(name="finp", bufs=3) as finp,
            tc.tile_pool(name="psum", bufs=8, space="PSUM") as psump,
        ):
            # ---- first x load + BN params go first so the first Sign
            # starts ASAP; weights overlap with it
            x_nats = {}

            def load_x(n):
                x_nats[n] = xp.tile([128, NHALF, S], fp32, name="x_nat",
                                    tag="x_nat")
                nc.sync.dma_start(x_nats[n][:], xin[n])

            load_x(0)
            sc_tile = constp.tile([128, NHALF], fp32)
            nc.sync.dma_start(sc_tile[:], bnsc.rearrange("h p -> p h"))
            bi_tile = constp.tile([128, NHALF], fp32)
            nc.sync.dma_start(bi_tile[:], bnbi.rearrange("h p -> p h"))
            w_tile = constp.tile([128, NHALF * K * K, 128], bf16)
            nc.sync.dma_start(w_tile[:], wT)

            # warmup: trigger the ACT table load early and keep the PE
            # HAM window busy so the real stream starts at full clock
            warm_sb = constp.tile([128, 512], bf16)
            nc.gpsimd.memset(warm_sb[:], 0.0)
            warm_w = constp.tile([128, 128], bf16)
            nc.gpsimd.memset(warm_w[:], 0.0)
            warm_act = constp.tile([128, 16], bf16)
            nc.scalar.activation(warm_act[:], warm_sb[:, 0:16],
                                 mybir.ActivationFunctionType.Sign)
            for _ in range(20):
                wps = psump.tile([128, 512], fp32, name="ps", tag="ps")
                nc.tensor.matmul(wps[:], warm_w[:], warm_sb[:],
                                 start=True, stop=True)

            # ---- persistent padded activation tiles, borders zeroed once
            a_pads = [[constp.tile([128, PH * PW], bf16,
                                   name=f"apad{h}_{b}")
                       for b in range(ABUFS)] for h in range(NHALF)]
            for h in range(NHALF):
                for b in range(ABUFS):
                    ap3 = a_pads[h][b][:].rearrange("p (y x) -> p y x", x=PW)
                    nc.gpsimd.memset(ap3[:, 0:PAD, :], 0.0)
                    nc.gpsimd.memset(ap3[:, PAD + H:PH, :], 0.0)
                    nc.gpsimd.memset(ap3[:, PAD:PAD + H, 0:PAD], 0.0)
                    nc.gpsimd.memset(ap3[:, PAD:PAD + H, PAD + W:PW], 0.0)

            # ---- remaining prefetches ----
            for n in range(1, min(PREFETCH, n_samples)):
                load_x(n)

            # deferred-by-one-sample ACT-side permutes + residual add +
            # store, so no engine FIFO makes sample n+1's work wait on
            # sample n's permute chain
            deferred = {}

            def perm_dma(eng, n, h, j):
                _, _, conv_i8_n = deferred[n]
                eng.dma_start(
                    conv_perm_of[n][64 * (j % 2) + 32 * h:
                                    64 * (j % 2) + 32 * h + 32,
                                    j // 2, :],
                    conv_i8_n[h][32 * j:32 * j + 32, :],
                )

            conv_perm_of = {}

            def finish_sample(n):
                x_nat_n, conv_perm_n, _ = deferred[n]
                for j in (2, 3):
                    for h in range(NHALF):
                        perm_dma(nc.scalar, n, h, j)
                fin = finp.tile([128, NHALF, S], fp32, name="fin",
                                tag="fin")
                for hh in range(NHALF):
                    nc.vector.tensor_add(
                        fin[:, hh, :], x_nat_n[:, hh, :],
                        conv_perm_n[:, hh, :])
                    nc.sync.dma_start(out[n][:, hh, :], fin[:, hh, :])
                deferred.pop(n)
                conv_perm_of.pop(n)

            for n in range(n_samples):
                if n + PREFETCH < n_samples:
                    load_x(n + PREFETCH)
                x_nat = x_nats.pop(n)

                # ---- a = Sign(x*scale + bias), bf16, into padded interior
                for h in range(NHALF):
                    ap3 = a_pads[h][n % ABUFS][:].rearrange(
                        "p (y x) -> p y x", x=PW)
                    nc.scalar.activation(
                        ap3[:, PAD:PAD + H, PAD:PAD + W],
                        x_nat[:, h, :].rearrange("p (y x) -> p y x", x=W),
                        mybir.ActivationFunctionType.Sign,
                        bias=bi_tile[:, h:h + 1],
                        scale=sc_tile[:, h:h + 1],
                    )

                # ---- conv: block-diag matmuls accumulating per chunk,
                # evicted to int8 (exact: conv values are small ints)
                conv_i8 = [ci8p.tile([128, S], i8, name=f"ci8_{h}",
                                     tag=f"ci8_{h}") for h in range(NHALF)]
                for h in range(NHALF):
                    ap3 = a_pads[h][n % ABUFS][:].rearrange(
                        "p (y x) -> p y x", x=PW)
                    for k in range(n_chunks):
                        ps = psump.tile([128, CHUNK_ROWS * W], fp32,
                                        name="ps", tag="ps")
                        for dy in range(K):
                            for dx in range(K):
                                t = dy * K + dx
                                nc.tensor.matmul(
                                    ps[:],
                                    w_tile[:, h * K * K + t, :],
                                    ap3[:,
                                        k * CHUNK_ROWS + DIL * dy:
                                        k * CHUNK_ROWS + DIL * dy
                                        + CHUNK_ROWS,
                                        DIL * dx:DIL * dx + W],
                                    start=(t == 0), stop=(t == K * K - 1),
                                )
                        lo = k * CHUNK_ROWS * W
                        nc.vector.tensor_copy(
                            conv_i8[h][:, lo:lo + CHUNK_ROWS * W], ps[:])

                # ---- shuffle-permute the int8 conv into natural final
                # channel order: psum (m=32j+g, half h) holds conv cout
                # 4g+j -> final channel 64j+32h+g = (slot j//2,
                # partition 64*(j%2)+32h+g).
                conv_perm = cpermp.tile([128, NHALF, S], i8)
                deferred[n] = (x_nat, conv_perm, conv_i8)
                conv_perm_of[n] = conv_perm
                for j in (0, 1):
                    for h in range(NHALF):
                        perm_dma(nc.gpsimd, n, h, j)

                # ---- rest of permute + add + store for PREVIOUS sample
                if n > 0:
                    finish_sample(n - 1)
            finish_sample(n_samples - 1)

    nc.compile()
    return nc


def _host_prep(x, weight, gamma, beta, running_mean, running_var):
    """Precompute BN affine + block-diagonal signed weights."""
    inv = (gamma / np.sqrt(running_var + EPS)).astype(np.float32)
    bias = (beta - running_mean * inv).astype(np.float32)
    wsign = np.sign(weight).astype(np.float32)   # [256, 4, 3, 3]

    lhsT = np.zeros((NHALF, K * K, 128, 128), np.float32)
    # Column m of lhsT (-> PSUM partition m) holds cout co = 4*(m%32)+m//32
    # within the half, so PSUM partition order is m = 32j + g for conv
    # cout 4g + j (see the device-side comment on conv_perm).
    m = np.arange(128)
    co = CPG * (m % 32) + m // 32
    gl = co // CPG
    for h in range(NHALF):
        for dy in range(K):
            for dx in range(K):
                t = dy * K + dx
                for kk in range(CPG):
                    lhsT[h, t, CPG * gl + kk, m] = wsign[128 * h + co, kk,
                                                         dy, dx]
    # device weight layout: [ci, (h,t), m] so the upload is contiguous
    lhsT = np.ascontiguousarray(
        lhsT.astype(ml_dtypes.bfloat16)
        .transpose(2, 0, 1, 3)
        .reshape(128, NHALF * K * K, 128))
    sc = np.ascontiguousarray(inv.reshape(NHALF, 128))
    bi = np.ascontiguousarray(bias.reshape(NHALF, 128))
    return lhsT, sc, bi


def _get_compiled():
    global _COMPILED
    if _COMPILED is None:
        _COMPILED = build(NS)
    return _COMPILED


def make_in_maps(x, weight, gamma, beta, running_mean, running_var):
    lhsT, sc, bi = _host_prep(x, weight, gamma, beta, running_mean,
                              running_var)
    # [cores, ns, 2, 128, S] -> partition-major [cores, ns, 128, 2, S]
    xs = np.ascontiguousarray(
        x.astype(np.float32)
        .reshape(N_CORES, NS, NHALF, 128, S)
        .transpose(0, 1, 3, 2, 4))
    return [
        {"xin": xs[i], "wT": lhsT, "bnsc": sc, "bnbi": bi}
        for i in range(N_CORES)
    ]


def kernel(x, weight, gamma, beta, running_mean, running_var):
    from concourse.bass_utils import run_bass_kernel_spmd

    nc = _get_compiled()
    in_maps = make_in_maps(np.asarray(x), np.asarray(weight),
                           np.asarray(gamma), np.asarray(beta),
                           np.asarray(running_mean), np.asarray(running_var))
    res = run_bass_kernel_spmd(nc, in_maps, list(range(N_CORES)))
    # device out is [ns, 128, 2, S] partition-major; channel c' = 128*slot+p
    outs = [res.results[i]["out"].transpose(0, 2, 1, 3).reshape(NS, C, H, W)
            for i in range(N_CORES)]
    return np.concatenate(outs, axis=0).astype(np.float32)


# revision 26
# speedup vs baseline: 1.4071x; 1.1366x over previous
"""BinaryDilGroupConv Trainium2 kernel.

Computes, for x[N=64, C=256, 32, 32]:
    h = BN(x)  (inference affine)
    a = sign(h); w = sign(weight)
    y = grouped dilated conv(a, w; groups=64, k=3, dil=2, pad=2)
    out = channel_shuffle(y, g=64) + x

Sharding: data-parallel over batch N across 8 NeuronCores (8 samples/core).
Params replicated. No collectives.

Device mapping (per core, per sample):
  - ACT: a = Sign(x*scale + bias) per 128-channel half, written bf16 into
    the interior of a zero-bordered padded tile [128, 36x36].
  - PE: grouped conv as block-diagonal matmuls: lhsT[h,tap] is [128 cin,
    128 cout] bf16 (zero off the 4x4 group diagonal). 9 dilation-shifted
    window reads of the padded tile accumulate into a PSUM tile per
    (half, 16-row chunk). PSUM partition order is m = 32j + g for conv
    cout 4g + j (lhsT columns permuted on the host) so the shuffle
    scatter below uses contiguous partition blocks.
  - DVE: evict PSUM to int8 (conv outputs are small integers, exact),
    DMA-permute the int8 conv tensor into shuffled (natural output)
    channel order (2MB instead of permuting 8MB of f32 x), then
    fin = x + conv_perm in natural layout.
  - Load and store are contiguous identity DMAs; the host pre/post
    reshapes (free) so DRAM runs are 8KB/partition.
"""

import numpy as np
import ml_dtypes

C = 256
G = 64            # groups
CPG = 4           # channels per group
K = 3
DIL = 2
PAD = 2
EPS = 1e-5
H = W = 32
S = H * W         # 1024 spatial positions
PH = H + 2 * PAD  # padded rows (36)
PW = W + 2 * PAD  # padded cols (36)
N_FULL = 64
N_CORES = 8
NS = N_FULL // N_CORES   # samples per core
NHALF = 2                # channel halves of 128
CHUNK_ROWS = 16          # psum chunk = 16 rows x 32 cols = 512 <= 1 bank
ABUFS = 3                # padded-activation round-robin depth

_COMPILED = None


def build(n_samples=NS):
    """Build + compile the per-core Bass program."""
    import concourse.bacc as bacc
    import concourse.tile as tile
    import concourse.mybir as mybir

    fp32 = mybir.dt.float32
    bf16 = mybir.dt.bfloat16
    i8 = mybir.dt.int8

    nc = bacc.Bacc("TRN2", target_bir_lowering=False, debug=False,
                   num_devices=N_CORES)

    # partition-major layouts so load/store DMAs are contiguous
    xin = nc.dram_tensor("xin", [n_samples, 128, NHALF, S], fp32,
                         kind="ExternalInput").ap()
    wT = nc.dram_tensor("wT", [128, NHALF * K * K, 128], bf16,
                        kind="ExternalInput").ap()
    bnsc = nc.dram_tensor("bnsc", [NHALF, 128], fp32,
                          kind="ExternalInput").ap()
    bnbi = nc.dram_tensor("bnbi", [NHALF, 128], fp32,
                          kind="ExternalInput").ap()
    out = nc.dram_tensor("out", [n_samples, 128, NHALF, S], fp32,
                         kind="ExternalOutput").ap()

    n_chunks = H // CHUNK_ROWS

    with tile.TileContext(nc) as tc:
        with (
            tc.tile_pool(name="const", bufs=1) as constp,
            tc.tile_pool(name="xp", bufs=n_samples) as xp,
            tc.tile_pool(name="ci8p", bufs=4) as ci8p,
            tc.tile_pool(name="cpermp", bufs=3) as cpermp,
            tc.tile_pool(name="finp", bufs=3) as finp,
            tc.tile_pool(name="psum", bufs=8, space="PSUM") as psump,
        ):
            # ---- first x load + BN params go first so the first Sign
            # starts ASAP; weights overlap with it
            x_nats = {}

            def load_x(n):
                x_nats[n] = xp.tile([128, NHALF, S], fp32, name="x_nat",
                                    tag="x_nat")
                nc.sync.dma_start(x_nats[n][:], xin[n])

            load_x(0)
            sc_tile = constp.tile([128, NHALF], fp32)
            nc.sync.dma_start(sc_tile[:], bnsc.rearrange("h p -> p h"))
            bi_tile = constp.tile([128, NHALF], fp32)
            nc.sync.dma_start(bi_tile[:], bnbi.rearrange("h p -> p h"))
            w_tile = constp.tile([128, NHALF * K * K, 128], bf16)
            nc.sync.dma_start(w_tile[:], wT)

            # warmup: trigger the ACT table load early and keep the PE
            # HAM window busy so the real stream starts at full clock
            warm_sb = constp.tile([128, 512], bf16)
            nc.gpsimd.memset(warm_sb[:], 0.0)
            warm_w = constp.tile([128, 128], bf16)
            nc.gpsimd.memset(warm_w[:], 0.0)
            warm_act = constp.tile([128, 16], bf16)
            nc.scalar.activation(warm_act[:], warm_sb[:, 0:16],
                                 mybir.ActivationFunctionType.Sign)
            for _ in range(20):
                wps = psump.tile([128, 512], fp32, name="ps", tag="ps")
                nc.tensor.matmul(wps[:], warm_w[:], warm_sb[:],
                                 start=True, stop=True)

            # ---- persistent padded activation tiles, borders zeroed once
            a_pads = [[constp.tile([128, PH * PW], bf16,
                                   name=f"apad{h}_{b}")
                       for b in range(ABUFS)] for h in range(NHALF)]
            for h in range(NHALF):
                for b in range(ABUFS):
                    ap3 = a_pads[h][b][:].rearrange("p (y x) -> p y x", x=PW)
                    nc.gpsimd.memset(ap3[:, 0:PAD, :], 0.0)
                    nc.gpsimd.memset(ap3[:, PAD + H:PH, :], 0.0)
                    nc.gpsimd.memset(ap3[:, PAD:PAD + H, 0:PAD], 0.0)
                    nc.gpsimd.memset(ap3[:, PAD:PAD + H, PAD + W:PW], 0.0)

            # ---- front-load ALL remaining x loads: the DMA engines run
            # at the edge of saturation during the matmul stream, so the
            # input traffic is moved to the (DMA-idle) prologue
            for n in range(1, n_samples):
                load_x(n)

            # deferred-by-one-sample ACT-side permutes + residual add +
            # store, so no engine FIFO makes sample n+1's work wait on
            # sample n's permute chain
            deferred = {}

            def perm_dma(eng, n, h, j):
                _, _, conv_i8_n = deferred[n]
                eng.dma_start(
                    conv_perm_of[n][64 * (j % 2) + 32 * h:
                                    64 * (j % 2) + 32 * h + 32,
                                    j // 2, :],
                    conv_i8_n[h][32 * j:32 * j + 32, :],
                )

            conv_perm_of = {}

            def finish_sample(n):
                x_nat_n, conv_perm_n, _ = deferred[n]
                for j in (2, 3):
                    for h in range(NHALF):
                        perm_dma(nc.scalar, n, h, j)
                fin = finp.tile([128, NHALF, S], fp32, name="fin",
                                tag="fin")
                for hh in range(NHALF):
                    nc.vector.tensor_add(
                        fin[:, hh, :], x_nat_n[:, hh, :],
                        conv_perm_n[:, hh, :])
                    nc.sync.dma_start(out[n][:, hh, :], fin[:, hh, :])
                deferred.pop(n)
                conv_perm_of.pop(n)

            for n in range(n_samples):
                x_nat = x_nats.pop(n)

                # ---- a = Sign(x*scale + bias), bf16, into padded interior
                for h in range(NHALF):
                    ap3 = a_pads[h][n % ABUFS][:].rearrange(
                        "p (y x) -> p y x", x=PW)
                    nc.scalar.activation(
                        ap3[:, PAD:PAD + H, PAD:PAD + W],
                        x_nat[:, h, :].rearrange("p (y x) -> p y x", x=W),
                        mybir.ActivationFunctionType.Sign,
                        bias=bi_tile[:, h:h + 1],
                        scale=sc_tile[:, h:h + 1],
                    )

                # ---- conv: block-diag matmuls accumulating per chunk,
                # evicted to int8 (exact: conv values are small ints)
                conv_i8 = [ci8p.tile([128, S], i8, name=f"ci8_{h}",
                                     tag=f"ci8_{h}") for h in range(NHALF)]
                for h in range(NHALF):
                    ap3 = a_pads[h][n % ABUFS][:].rearrange(
                        "p (y x) -> p y x", x=PW)
                    for k in range(n_chunks):
                        ps = psump.tile([128, CHUNK_ROWS * W], fp32,
                                        name="ps", tag="ps")
                        for dy in range(K):
                            for dx in range(K):
                                t = dy * K + dx
                                nc.tensor.matmul(
                                    ps[:],
                                    w_tile[:, h * K * K + t, :],
                                    ap3[:,
                                        k * CHUNK_ROWS + DIL * dy:
                                        k * CHUNK_ROWS + DIL * dy
                                        + CHUNK_ROWS,
                                        DIL * dx:DIL * dx + W],
                                    start=(t == 0), stop=(t == K * K - 1),
                                )
                        lo = k * CHUNK_ROWS * W
                        nc.vector.tensor_copy(
                            conv_i8[h][:, lo:lo + CHUNK_ROWS * W], ps[:])

                # ---- shuffle-permute the int8 conv into natural final
                # channel order: psum (m=32j+g, half h) holds conv cout
                # 4g+j -> final channel 64j+32h+g = (slot j//2,
                # partition 64*(j%2)+32h+g).
                conv_perm = cpermp.tile([128, NHALF, S], i8)
                deferred[n] = (x_nat, conv_perm, conv_i8)
                conv_perm_of[n] = conv_perm
                for j in (0, 1):
                    for h in range(NHALF):
                        perm_dma(nc.gpsimd, n, h, j)

                # ---- rest of permute + add + store for PREVIOUS sample
                if n > 0:
                    finish_sample(n - 1)
            finish_sample(n_samples - 1)

    nc.compile()
    return nc


def _host_prep(x, weight, gamma, beta, running_mean, running_var):
    """Precompute BN affine + block-diagonal signed weights."""
    inv = (gamma / np.sqrt(running_var + EPS)).astype(np.float32)
    bias = (beta - running_mean * inv).astype(np.float32)
    wsign = np.sign(weight).astype(np.float32)   # [256, 4, 3, 3]

    lhsT = np.zeros((NHALF, K * K, 128, 128), np.float32)
    # Column m of lhsT (-> PSUM partition m) holds cout co = 4*(m%32)+m//32
    # within the half, so PSUM partition order is m = 32j + g for conv
    # cout 4g + j (see the device-side comment on conv_perm).
    m = np.arange(128)
    co = CPG * (m % 32) + m // 32
    gl = co // CPG
    for h in range(NHALF):
        for dy in range(K):
            for dx in range(K):
                t = dy * K + dx
                for kk in range(CPG):
                    lhsT[h, t, CPG * gl + kk, m] = wsign[128 * h + co, kk,
                                                         dy, dx]
    # device weight layout: [ci, (h,t), m] so the upload is contiguous
    lhsT = np.ascontiguousarray(
        lhsT.astype(ml_dtypes.bfloat16)
        .transpose(2, 0, 1, 3)
        .reshape(128, NHALF * K * K, 128))
    sc = np.ascontiguousarray(inv.reshape(NHALF, 128))
    bi = np.ascontiguousarray(bias.reshape(NHALF, 128))
    return lhsT, sc, bi


def _get_compiled():
    global _COMPILED
    if _COMPILED is None:
        _COMPILED = build(NS)
    return _COMPILED


def make_in_maps(x, weight, gamma, beta, running_mean, running_var):
    lhsT, sc, bi = _host_prep(x, weight, gamma, beta, running_mean,
                              running_var)
    # [cores, ns, 2, 128, S] -> partition-major [cores, ns, 128, 2, S]
    xs = np.ascontiguousarray(
        x.astype(np.float32)
        .reshape(N_CORES, NS, NHALF, 128, S)
        .transpose(0, 1, 3, 2, 4))
    return [
        {"xin": xs[i], "wT": lhsT, "bnsc": sc, "bnbi": bi}
        for i in range(N_CORES)
    ]


def kernel(x, weight, gamma, beta, running_mean, running_var):
    from concourse.bass_utils import run_bass_kernel_spmd

    nc = _get_compiled()
    in_maps = make_in_maps(np.asarray(x), np.asarray(weight),
                           np.asarray(gamma), np.asarray(beta),
                           np.asarray(running_mean), np.asarray(running_var))
    res = run_bass_kernel_spmd(nc, in_maps, list(range(N_CORES)))
    # device out is [ns, 128, 2, S] partition-major; channel c' = 128*slot+p
    outs = [res.results[i]["out"].transpose(0, 2, 1, 3).reshape(NS, C, H, W)
            for i in range(N_CORES)]
    return np.concatenate(outs, axis=0).astype(np.float32)


# revision 28
# speedup vs baseline: 1.5770x; 1.1207x over previous
"""BinaryDilGroupConv Trainium2 kernel.

Computes, for x[N=64, C=256, 32, 32]:
    h = BN(x)  (inference affine)
    a = sign(h); w = sign(weight)
    y = grouped dilated conv(a, w; groups=64, k=3, dil=2, pad=2)
    out = channel_shuffle(y, g=64) + x

Sharding: data-parallel over batch N across 8 NeuronCores (8 samples/core).
Params replicated. No collectives.

Device mapping (per core, per sample):
  - ACT: a = Sign(x*scale + bias) per 128-channel half, written fp8 into
    the interior of a zero-bordered padded tile (row pitch 40 bytes).
  - PE: grouped conv as block-diagonal matmuls: lhsT is [128 cin, 128
    cout] fp8 (zero off the 4x4 group diagonal), dilation handled by
    flat shifted-window reads of the padded tile. The dy=0/dy=1 tap
    pairs run as fp8 DoubleRow matmuls (pair stride 2 rows = 80B), the
    dy=2 taps as plain fp8 matmuls; 6 matmuls accumulate per PSUM tile
    of ny x 40 columns (x >= 32 columns are discarded as junk). PSUM
    partition order is m = 32j + g for conv cout 4g + j (lhsT columns
    permuted on the host) so the shuffle below uses contiguous blocks.
  - DVE: evict PSUM (keeping x < 32) to int8 (conv outputs are small
    integers, exact), DMA-permute the int8 conv tensor into shuffled
    (natural output) channel order, then fin = x + conv_perm.
  - Load and store are contiguous identity DMAs; the host pre/post
    reshapes (free) so DRAM runs are 8KB/partition.
"""

import numpy as np
import ml_dtypes

C = 256
G = 64            # groups
CPG = 4           # channels per group
K = 3
DIL = 2
PAD = 2
EPS = 1e-5
H = W = 32
S = H * W         # 1024 spatial positions
PH = 38           # padded rows (36 used + 2 spill rows for flat windows)
PW = 40           # padded cols (36 used + 4: row pitch 40B makes the
                  # DoubleRow pair stride 80B, a multiple of 16)
N_FULL = 64
N_CORES = 8
NS = N_FULL // N_CORES   # samples per core
NHALF = 2                # channel halves of 128
CHUNKS = [(0, 12), (12, 12), (24, 8)]  # (y0, ny): ny*40 <= 512 psum bank
ABUFS = 3                # padded-activation round-robin depth

_COMPILED = None


def build(n_samples=NS):
    """Build + compile the per-core Bass program."""
    import concourse.bass as bass
    import concourse.bacc as bacc
    import concourse.tile as tile
    import concourse.mybir as mybir

    fp32 = mybir.dt.float32
    fp8 = mybir.dt.float8e4
    i8 = mybir.dt.int8

    nc = bacc.Bacc("TRN2", target_bir_lowering=False, debug=False,
                   num_devices=N_CORES)

    # partition-major layouts so load/store DMAs are contiguous
    xin = nc.dram_tensor("xin", [n_samples, 128, NHALF, S], fp32,
                         kind="ExternalInput").ap()
    # weight free index = h*9 + dx*3 + slot (slot 0/1 = dy 0/1 pair
    # members, slot 2 = dy 2 single)
    wT = nc.dram_tensor("wT", [128, NHALF * K * K, 128], fp8,
                        kind="ExternalInput").ap()
    bnsc = nc.dram_tensor("bnsc", [NHALF, 128], fp32,
                          kind="ExternalInput").ap()
    bnbi = nc.dram_tensor("bnbi", [NHALF, 128], fp32,
                          kind="ExternalInput").ap()
    out = nc.dram_tensor("out", [n_samples, 128, NHALF, S], fp32,
                         kind="ExternalOutput").ap()

    with tile.TileContext(nc) as tc:
        with (
            tc.tile_pool(name="const", bufs=1) as constp,
            tc.tile_pool(name="xp", bufs=n_samples) as xp,
            tc.tile_pool(name="ci8p", bufs=4) as ci8p,
            tc.tile_pool(name="cpermp", bufs=3) as cpermp,
            tc.tile_pool(name="finp", bufs=3) as finp,
            tc.tile_pool(name="psum", bufs=8, space="PSUM") as psump,
        ):
            # ---- first x load + BN params go first so the first Sign
            # starts ASAP; weights overlap with it
            x_nats = {}

            def load_x(n):
                x_nats[n] = xp.tile([128, NHALF, S], fp32, name="x_nat",
                                    tag="x_nat")
                nc.sync.dma_start(x_nats[n][:], xin[n])

            load_x(0)
            sc_tile = constp.tile([128, NHALF], fp32)
            nc.sync.dma_start(sc_tile[:], bnsc.rearrange("h p -> p h"))
            bi_tile = constp.tile([128, NHALF], fp32)
            nc.sync.dma_start(bi_tile[:], bnbi.rearrange("h p -> p h"))
            w_tile = constp.tile([128, NHALF * K * K, 128], fp8)
            nc.sync.dma_start(w_tile[:], wT)

            # warmup: trigger the ACT table load early and keep the PE
            # HAM window busy until the real stream starts. The second
            # batch reads the real weight tile so it runs right before
            # the first real matmul (bridging the HAM activity window).
            warm_sb = constp.tile([128, 512], fp8)
            nc.gpsimd.memset(warm_sb[:], 0.0)
            warm_w = constp.tile([128, 128], fp8)
            nc.gpsimd.memset(warm_w[:], 0.0)
            warm_act = constp.tile([128, 16], fp8)
            nc.scalar.activation(warm_act[:], warm_sb[:, 0:16],
                                 mybir.ActivationFunctionType.Sign)
            for _ in range(12):
                wps = psump.tile([128, 512], fp32, name="ps", tag="ps")
                nc.tensor.matmul(wps[:], warm_w[:], warm_sb[:],
                                 start=True, stop=True)
            for _ in range(8):
                wps = psump.tile([128, 512], fp32, name="ps", tag="ps")
                nc.tensor.matmul(wps[:], w_tile[:, 0, :], warm_sb[:],
                                 start=True, stop=True)

            # ---- persistent padded activation tiles, borders zeroed once
            a_pads = [[constp.tile([128, PH * PW], fp8,
                                   name=f"apad{h}_{b}")
                       for b in range(ABUFS)] for h in range(NHALF)]
            for h in range(NHALF):
                for b in range(ABUFS):
                    ap3 = a_pads[h][b][:].rearrange("p (y x) -> p y x", x=PW)
                    nc.gpsimd.memset(ap3[:, 0:PAD, :], 0.0)
                    nc.gpsimd.memset(ap3[:, PAD + H:PH, :], 0.0)
                    nc.gpsimd.memset(ap3[:, PAD:PAD + H, 0:PAD], 0.0)
                    nc.gpsimd.memset(ap3[:, PAD:PAD + H, PAD + W:PW], 0.0)

            # ---- front-load ALL remaining x loads: the DMA engines run
            # at the edge of saturation during the matmul stream, so the
            # input traffic is moved to the (DMA-idle) prologue
            for n in range(1, n_samples):
                load_x(n)

            # deferred-by-one-sample ACT-side permutes + residual add +
            # store, so no engine FIFO makes sample n+1's work wait on
            # sample n's permute chain
            deferred = {}
            conv_perm_of = {}

            def perm_dma(eng, n, h, j):
                _, _, conv_i8_n = deferred[n]
                eng.dma_start(
                    conv_perm_of[n][64 * (j % 2) + 32 * h:
                                    64 * (j % 2) + 32 * h + 32,
                                    j // 2, :],
                    conv_i8_n[h][32 * j:32 * j + 32, :],
                )

            def finish_sample(n):
                x_nat_n, conv_perm_n, _ = deferred[n]
                for j in (2, 3):
                    for h in range(NHALF):
                        perm_dma(nc.scalar, n, h, j)
                fin = finp.tile([128, NHALF, S], fp32, name="fin",
                                tag="fin")
                for hh in range(NHALF):
                    nc.vector.tensor_add(
                        fin[:, hh, :], x_nat_n[:, hh, :],
                        conv_perm_n[:, hh, :])
                    nc.sync.dma_start(out[n][:, hh, :], fin[:, hh, :])
                deferred.pop(n)
                conv_perm_of.pop(n)

            def window(apad, offset, rsteps, ncols):
                """Flat shifted-window AP [128, rsteps?, ncols] of the
                padded activation tile (manual AP: the pair dim strides
                2 rows = 80 elements, not expressible by rearrange)."""
                base = apad[:, offset:offset + 1]
                ap = [list(apad[:].ap[0])]
                if rsteps:
                    ap.append([2 * PW, rsteps])
                ap.append([1, ncols])
                return bass.AP(base.tensor, base.offset, ap)

            for n in range(n_samples):
                x_nat = x_nats.pop(n)

                # ---- a = Sign(x*scale + bias), fp8, into padded interior
                for h in range(NHALF):
                    ap3 = a_pads[h][n % ABUFS][:].rearrange(
                        "p (y x) -> p y x", x=PW)
                    nc.scalar.activation(
                        ap3[:, PAD:PAD + H, PAD:PAD + W],
                        x_nat[:, h, :].rearrange("p (y x) -> p y x", x=W),
                        mybir.ActivationFunctionType.Sign,
                        bias=bi_tile[:, h:h + 1],
                        scale=sc_tile[:, h:h + 1],
                    )

                # ---- conv: fp8 DoubleRow pairs + singles per chunk,
                # evicted (x < 32 only) to int8
                conv_i8 = [ci8p.tile([128, S], i8, name=f"ci8_{h}",
                                     tag=f"ci8_{h}") for h in range(NHALF)]
                for h in range(NHALF):
                    apad = a_pads[h][n % ABUFS]
                    for (y0, ny) in CHUNKS:
                        N = ny * PW
                        ps = psump.tile([128, N], fp32, name="ps", tag="ps")
                        for dx in range(K):
                            wi = h * K * K + dx * K
                            nc.tensor.matmul(
                                ps[:],
                                w_tile[:, wi:wi + 2, :],
                                window(apad, y0 * PW + DIL * dx, 2, N),
                                start=(dx == 0), stop=False,
                                perf_mode=mybir.MatmulPerfMode.DoubleRow,
                            )
                        for dx in range(K):
                            wi = h * K * K + dx * K + 2
                            nc.tensor.matmul(
                                ps[:],
                                w_tile[:, wi, :],
                                window(apad, (y0 + 2 * DIL) * PW + DIL * dx,
                                       0, N),
                                start=False, stop=(dx == K - 1),
                            )
                        nc.vector.tensor_copy(
                            conv_i8[h][:, y0 * W:(y0 + ny) * W].rearrange(
                                "p (y x) -> p y x", x=W),
                            ps[:].rearrange("p (y x) -> p y x",
                                            x=PW)[:, :, 0:W],
                        )

                # ---- shuffle-permute the int8 conv into natural final
                # channel order: psum (m=32j+g, half h) holds conv cout
                # 4g+j -> final channel 64j+32h+g = (slot j//2,
                # partition 64*(j%2)+32h+g).
                conv_perm = cpermp.tile([128, NHALF, S], i8)
                deferred[n] = (x_nat, conv_perm, conv_i8)
                conv_perm_of[n] = conv_perm
                for j in (0, 1):
                    for h in range(NHALF):
                        perm_dma(nc.gpsimd, n, h, j)

                # ---- rest of permute + add + store for PREVIOUS sample
                if n > 0:
                    finish_sample(n - 1)
            finish_sample(n_samples - 1)

    nc.compile()
    return nc


def _host_prep(x, weight, gamma, beta, running_mean, running_var):
    """Precompute BN affine + block-diagonal signed weights."""
    inv = (gamma / np.sqrt(running_var + EPS)).astype(np.float32)
    bias = (beta - running_mean * inv).astype(np.float32)
    wsign = np.sign(weight).astype(np.float32)   # [256, 4, 3, 3]

    lhsT = np.zeros((NHALF, K * K, 128, 128), np.float32)
    # Column m of lhsT (-> PSUM partition m) holds cout co = 4*(m%32)+m//32
    # within the half, so PSUM partition order is m = 32j + g for conv
    # cout 4g + j (see the device-side comment on conv_perm).
    m = np.arange(128)
    co = CPG * (m % 32) + m // 32
    gl = co // CPG
    for h in range(NHALF):
        for dy in range(K):
            for dx in range(K):
                # device tap index: dx*3 + dy (dy 0/1 = DoubleRow pair)
                t = dx * K + dy
                for kk in range(CPG):
                    lhsT[h, t, CPG * gl + kk, m] = wsign[128 * h + co, kk,
                                                         dy, dx]
    # device weight layout: [ci, (h,t), m], fp8, contiguous upload
    lhsT = np.ascontiguousarray(
        lhsT.astype(ml_dtypes.float8_e4m3)
        .transpose(2, 0, 1, 3)
        .reshape(128, NHALF * K * K, 128))
    sc = np.ascontiguousarray(inv.reshape(NHALF, 128))
    bi = np.ascontiguousarray(bias.reshape(NHALF, 128))
    return lhsT, sc, bi


def _get_compiled():
    global _COMPILED
    if _COMPILED is None:
        _COMPILED = build(NS)
    return _COMPILED


def make_in_maps(x, weight, gamma, beta, running_mean, running_var):
    lhsT, sc, bi = _host_prep(x, weight, gamma, beta, running_mean,
                              running_var)
    # [cores, ns, 2, 128, S] -> partition-major [cores, ns, 128, 2, S]
    xs = np.ascontiguousarray(
        x.astype(np.float32)
        .reshape(N_CORES, NS, NHALF, 128, S)
        .transpose(0, 1, 3, 2, 4))
    return [
        {"xin": xs[i], "wT": lhsT, "bnsc": sc, "bnbi": bi}
        for i in range(N_CORES)
    ]


def kernel(x, weight, gamma, beta, running_mean, running_var):
    from concourse.bass_utils import run_bass_kernel_spmd

    nc = _get_compiled()
    in_maps = make_in_maps(np.asarray(x), np.asarray(weight),
                           np.asarray(gamma), np.asarray(beta),
                           np.asarray(running_mean), np.asarray(running_var))
    res = run_bass_kernel_spmd(nc, in_maps, list(range(N_CORES)))
    # device out is [ns, 128, 2, S] partition-major; channel c' = 128*slot+p
    outs = [res.results[i]["out"].transpose(0, 2, 1, 3).reshape(NS, C, H, W)
            for i in range(N_CORES)]
    return np.concatenate(outs, axis=0).astype(np.float32)
